# revision 8
# baseline (speedup 1.0000x reference)
"""MoE feed-forward kernel for Trainium2 (8 NeuronCores, SPMD expert-parallel).

Strategy
--------
Host side (inside kernel()):
  * Compute the MoE gate (softmax + top-2 + renormalize) in float64.
  * Expert pairing: sort experts by token count, pair largest with
    smallest; each core gets two routed blocks (B1 + B2 columns, one
    per expert of its pair), splitting each pair's tokens evenly
    across its two cores.  This cuts SPMD padding from max(count) to
    ~(count_i+count_j)/2 per core.
  * Shared expert is sharded 2D: token-quarter (c % 4) x F-half (c // 4).
  * Weights/activations are packed to bf16 in matmul-native layouts.
Device side (one Bass/Tile program, run on all 8 cores with different data):
  * up/gate:  uT[f,:] = sum_k wug[k,f].T @ xT[k,:]   (F on partitions)
  * a = silu(u) * g  (ACT + DVE), kept bf16 in SBUF
  * down (transposed): y[h_tile,:] = sum_f wdT[f,h].T @ aT[f][:, cols]
    -- token columns moving, so no 128-row padding; routing weight is
    applied to the PSUM result via a per-column broadcast multiply.
  * host scatter-adds outputs (f32).
"""

import os
import numpy as np
import ml_dtypes

import concourse.bacc as bacc
import concourse.mybir as mybir
import concourse.tile as tile
from concourse.bass_utils import run_bass_kernel_spmd

BF16 = mybir.dt.bfloat16
F32 = mybir.dt.float32
P = 128

# Problem dims (hardcoded per contest rules; kernel.py must be self-contained).
H = 2048
F = 5632
E = 8
TOP_K = 2
T = 2048
N_CORES = 8
KT = H // P          # 16 k-tiles (contraction over H)
FT = F // P          # 44 f-tiles
FS = FT // 2         # 22 f-tiles for the shared F-half
HT = H // P          # 16 h-tiles (down-projection output)
ST = T // 4          # 512 shared tokens per core (token quarter)

LAST_EXEC_NS = None
LAST_RESULTS = None

_compiled = {}


def _build(bsizes, wslots):
    """Build + compile the SPMD Bass program.

    bsizes: per-routed-block column counts (each <= 512).
    wslots: weight-slot index each block reads (nw = max+1 slots).
    """
    C = sum(bsizes)
    nw = max(wslots) + 1
    blocks = []
    off = 0
    for bs, ws in zip(bsizes, wslots):
        blocks.append((off, bs, ws))
        off += bs

    nc = bacc.Bacc(
        "TRN2",
        target_bir_lowering=False,
        debug=False,
        enable_asserts=False,
        num_devices=N_CORES,
    )

    xe_d = nc.dram_tensor("xe", [P, KT, C], BF16, kind="ExternalInput")
    xs_d = nc.dram_tensor("xs", [P, KT, ST], BF16, kind="ExternalInput")
    rwb_d = nc.dram_tensor("rwb", [P, C], F32, kind="ExternalInput")
    wug_d = nc.dram_tensor("wug", [P, nw, FT, 2, KT, P], BF16, kind="ExternalInput")
    wdt_d = nc.dram_tensor("wdt", [P, nw, HT, FT, P], BF16, kind="ExternalInput")
    sug_d = nc.dram_tensor("sug", [P, 1, FS, 2, KT, P], BF16, kind="ExternalInput")
    sdt_d = nc.dram_tensor("sdt", [P, HT, FS, P], BF16, kind="ExternalInput")
    ye_d = nc.dram_tensor("ye", [HT, P, C], F32, kind="ExternalOutput")
    ys_d = nc.dram_tensor("ys", [HT, P, ST], F32, kind="ExternalOutput")

    with tile.TileContext(nc) as tc:
        with (
            tc.tile_pool(name="const", bufs=1) as cpool,
            tc.tile_pool(name="acts", bufs=1) as apool,
            tc.tile_pool(name="wug_s", bufs=2 * nw) as wpool,
            tc.tile_pool(name="wdt_s", bufs=2 * nw) as wdpool,
            tc.tile_pool(name="sdt_s", bufs=2) as sdpool,
            tc.tile_pool(name="tmp", bufs=2) as tpool,
            tc.tile_pool(name="osb", bufs=4) as opool,
            tc.tile_pool(name="ps_a", bufs=2, space="PSUM") as pa_pool,
            tc.tile_pool(name="ps_b", bufs=2, space="PSUM") as pb_pool,
            tc.tile_pool(name="ps_c", bufs=2, space="PSUM") as pc_pool,
            tc.tile_pool(name="ps_d", bufs=2, space="PSUM") as pd_pool,
        ):
            # Startup-critical DMAs first: the fi=0 weight tiles, then the
            # routed activations in k-groups so the first matmul can begin
            # as soon as w(fi=0) and xe[k=0] land.
            # The very first matmul chain only needs the "up" plane of the
            # slot-0 fi=0 weight tile plus xe[k=0]; order DMAs accordingly.
            w0 = []
            for s in range(nw):
                w = wpool.tile([P, 2, KT, P], BF16, tag="wug", name=f"w_r0_{s}")
                w0.append(w)
            xe_sb = cpool.tile([P, KT, C], BF16, tag="xe", name="xe_sb")
            nc.sync.dma_start(w0[0][:, 0:1], wug_d[:, 0, 0, 0:1])
            nc.sync.dma_start(xe_sb[:, 0:2], xe_d[:, 0:2])
            for s in range(1, nw):
                nc.sync.dma_start(w0[s][:, 0:1], wug_d[:, s, 0, 0:1])
            nc.sync.dma_start(xe_sb[:, 2:6], xe_d[:, 2:6])
            for s in range(nw):
                nc.sync.dma_start(w0[s][:, 1:2], wug_d[:, s, 0, 1:2])
            for k0, k1 in ((6, 11), (11, KT)):
                nc.sync.dma_start(xe_sb[:, k0:k1], xe_d[:, k0:k1])

            # phase-1/2 PSUM mapping: block0 -> (pa, pc); block1 -> (pb, pd)
            pools_u = [pa_pool, pb_pool]
            pools_g = [pc_pool, pd_pool]

            def up_gate(n_ft, w_dram, x_sb, blks, ctot, out_tag, w_pre=None):
                outs = []
                for fi in range(n_ft):
                    if fi == 0 and w_pre is not None:
                        ws = w_pre
                    else:
                        ws = []
                        for s in range(w_dram.shape[1]):
                            w = wpool.tile([P, 2, KT, P], BF16, tag="wug",
                                           name=f"w_{out_tag}_{fi}_{s}")
                            nc.sync.dma_start(w[:], w_dram[:, s, fi])
                            ws.append(w)
                    a_f = apool.tile([P, ctot], BF16, tag=out_tag, bufs=n_ft,
                                     name=f"a_{out_tag}_{fi}")
                    for bi, (boff, bs, slot) in enumerate(blks):
                        pu = pools_u[bi].tile([P, 512], F32, tag=f"pu{bi}",
                                              name=f"pu_{out_tag}_{fi}_{bi}")
                        pg = pools_g[bi].tile([P, 512], F32, tag=f"pg{bi}",
                                              name=f"pg_{out_tag}_{fi}_{bi}")
                        for k in range(KT):
                            nc.tensor.matmul(pu[:, :bs], ws[slot][:, 0, k],
                                             x_sb[:, k, boff:boff + bs],
                                             start=(k == 0), stop=(k == KT - 1))
                        for k in range(KT):
                            nc.tensor.matmul(pg[:, :bs], ws[slot][:, 1, k],
                                             x_sb[:, k, boff:boff + bs],
                                             start=(k == 0), stop=(k == KT - 1))
                        su = tpool.tile([P, 512], F32, tag="su",
                                        name=f"su_{out_tag}_{fi}_{bi}")
                        nc.scalar.activation(su[:, :bs], pu[:, :bs],
                                             mybir.ActivationFunctionType.Sigmoid)
                        nc.vector.tensor_mul(su[:, :bs], su[:, :bs], pu[:, :bs])
                        nc.vector.tensor_mul(a_f[:, boff:boff + bs], su[:, :bs],
                                             pg[:, :bs])
                    outs.append(a_f)
                return outs

            aT = up_gate(FT, wug_d, xe_sb, blocks, C, "aT", w_pre=w0)

            # Shared-expert inputs stream in behind the routed phase.
            xs_sb = cpool.tile([P, KT, ST], BF16, tag="xs", name="xs_sb")
            nc.sync.dma_start(xs_sb[:], xs_d[:])
            rwb_sb = cpool.tile([P, C], F32, tag="rwb", name="rwb_sb")
            nc.sync.dma_start(rwb_sb[:], rwb_d[:])

            as2 = up_gate(FS, sug_d, xs_sb, [(0, ST, 0)], ST, "as2")

            # Merged down phase (transposed): per h-tile, contract over f
            # with token columns moving; routed blocks then the shared half.
            for ht in range(HT):
                wdt_t = []
                for s in range(nw):
                    wt = wdpool.tile([P, FT, P], BF16, tag="wdt",
                                     name=f"wdt_{ht}_{s}")
                    nc.sync.dma_start(wt[:], wdt_d[:, s, ht])
                    wdt_t.append(wt)
                sdt_t = sdpool.tile([P, FS, P], BF16, tag="sdt", name=f"sdt_{ht}")
                nc.sync.dma_start(sdt_t[:], sdt_d[:, ht])

                pys = []
                for bi, (boff, bs, slot) in enumerate(blocks):
                    py = pools_u[bi].tile([P, 512], F32, tag=f"pu{bi}",
                                          name=f"py_{ht}_{bi}")
                    pys.append(py)
                for fi in range(FT):
                    for bi, (boff, bs, slot) in enumerate(blocks):
                        nc.tensor.matmul(pys[bi][:, :bs], wdt_t[slot][:, fi],
                                         aT[fi][:, boff:boff + bs],
                                         start=(fi == 0), stop=(fi == FT - 1))
                for bi, (boff, bs, slot) in enumerate(blocks):
                    o = opool.tile([P, 512], F32, tag="o", name=f"o_{ht}_{bi}")
                    nc.vector.tensor_mul(o[:, :bs], pys[bi][:, :bs],
                                         rwb_sb[:, boff:boff + bs])
                    nc.sync.dma_start(ye_d[ht, :, boff:boff + bs], o[:, :bs])

                ps = pools_g[0].tile([P, 512], F32, tag="pg0", name=f"ps_{ht}")
                for fi in range(FS):
                    nc.tensor.matmul(ps[:, :ST], sdt_t[:, fi], as2[fi][:],
                                     start=(fi == 0), stop=(fi == FS - 1))
                o2 = opool.tile([P, 512], F32, tag="o", name=f"os_{ht}")
                nc.vector.tensor_copy(o2[:, :ST], ps[:, :ST])
                nc.sync.dma_start(ys_d[ht], o2[:, :ST])

    nc.compile()
    return nc


def _pack_ug(wu, wg):
    """[H, Fp] x2 (f32) -> [P, ftp, 2, KT, P] bf16."""
    kt = wu.shape[0] // P
    ftp = wu.shape[1] // P
    ru = wu.reshape(kt, P, ftp, P).transpose(1, 2, 0, 3)
    rg = wg.reshape(kt, P, ftp, P).transpose(1, 2, 0, 3)
    return np.ascontiguousarray(
        np.stack([ru, rg], axis=2)).astype(ml_dtypes.bfloat16)


def _pack_downT(wd):
    """[Fp, H] f32 -> [P, HT, ftp, P] bf16 (transposed-down layout)."""
    fp, h = wd.shape
    ftp = fp // P
    r = wd.reshape(ftp, P, HT, P).transpose(1, 2, 0, 3)
    return np.ascontiguousarray(r).astype(ml_dtypes.bfloat16)


def _pack_xT(xrows):
    """[n, H] f32 -> [P, KT, n] bf16."""
    n, h = xrows.shape
    kt = h // P
    return np.ascontiguousarray(
        xrows.reshape(n, kt, P).transpose(2, 1, 0)).astype(ml_dtypes.bfloat16)


def _try_install_ntff_shim():
    """Register the NTFF profile hook that this container's antenv lacks,
    so run_bass_kernel_spmd(trace=True) can capture HW exec time."""
    try:
        import sys
        import types

        if "antenv.axon_hooks" not in sys.modules:
            import trn_agent_boot.trn_boot as tb

            hook = tb._ntff_profile_via_ctypes("/opt/axon/libaxon_pjrt.so")
            if hook is None:
                return False
            mod = types.ModuleType("antenv.axon_hooks")
            mod.get_axon_ntff_profile_hook = lambda: hook
            mod.set_axon_ntff_profile_hook = lambda h: None
            sys.modules["antenv.axon_hooks"] = mod
        import concourse.bass_utils as bu

        bu.upload_artifacts = lambda tmpdir: f"file://{tmpdir}"
        return True
    except Exception as e:  # pragma: no cover - profiling is best-effort
        print("ntff shim unavailable:", e)
        return False


def _gate(x, gate_w):
    """Host gate: float64 softmax + stable top-2 + renormalize."""
    logits = x.astype(np.float64) @ gate_w.T.astype(np.float64)
    logits -= logits.max(axis=-1, keepdims=True)
    ex = np.exp(logits)
    score = ex / ex.sum(axis=-1, keepdims=True)
    top2 = np.argsort(-score, axis=-1, kind="stable")[:, :TOP_K]
    tw = np.take_along_axis(score, top2, axis=-1)
    tw = tw / (tw.sum(axis=-1, keepdims=True) + 1e-20)
    return top2, tw


def _plan_blocks(counts):
    """Pair experts (largest with smallest) and choose uniform block sizes.

    Returns (B1, B2, assignments) where assignments[core] =
    [(expert, lo, hi), (expert, lo, hi)] token ranges per block, or None
    if pairing is infeasible (fall back to single-expert blocks).
    """
    order = np.argsort(-counts, kind="stable")
    pairs = [(int(order[m]), int(order[N_CORES - 1 - m])) for m in range(N_CORES // 2)]
    B1 = int(np.ceil(counts.max() / 2 / 2) * 2)
    small_max = max(int(counts[j]) for _, j in pairs)
    B2 = int(np.ceil(small_max / 2 / 2) * 2)

    for _ in range(600):
        splits = []
        ok = True
        for i, j in pairs:
            ni, nj = int(counts[i]), int(counts[j])
            target = (ni + nj + 1) // 2
            xlo = max(0, ni - B1, target - B2, target - nj)
            xhi = min(B1, ni, target, target - max(0, nj - B2))
            if xlo > xhi or B1 + B2 < target:
                ok = False
                break
            x = xhi
            y = target - x
            splits.append((x, y))
        if ok:
            break
        B2 += 2
    else:
        return None

    assignments = []
    for m, ((i, j), (x, y)) in enumerate(zip(pairs, splits)):
        ni, nj = int(counts[i]), int(counts[j])
        assignments.append([(i, 0, x), (j, 0, y)])
        assignments.append([(i, x, ni), (j, y, nj)])
    return B1, B2, assignments


def kernel(hidden_state, gate_w, w_gate, w_up, w_down, sw_gate, sw_up, sw_down):
    global LAST_EXEC_NS, LAST_RESULTS

    x = np.asarray(hidden_state, dtype=np.float32).reshape(-1, H)
    gate_w = np.asarray(gate_w, dtype=np.float32)
    w_gate = np.asarray(w_gate, dtype=np.float32)
    w_up = np.asarray(w_up, dtype=np.float32)
    w_down = np.asarray(w_down, dtype=np.float32)
    sw_gate = np.asarray(sw_gate, dtype=np.float32)
    sw_up = np.asarray(sw_up, dtype=np.float32)
    sw_down = np.asarray(sw_down, dtype=np.float32)

    top2, tw = _gate(x, gate_w)

    idx_e, w_e = [], []
    for e in range(E):
        sel = top2 == e
        rows = np.flatnonzero(sel.any(axis=1))
        ww = (tw * sel)[rows].sum(axis=1)
        idx_e.append(rows)
        w_e.append(ww.astype(np.float32))
    counts = np.array([len(i) for i in idx_e])

    plan = _plan_blocks(counts)
    if plan is not None:
        B1, B2, assignments = plan
        bsizes, wslots = (B1, B2), (0, 1)
    else:
        # Degenerate fallback: one expert per core, single weight slot.
        Cmax = max(int(np.ceil(counts.max() / 8)) * 8, P)
        b1 = min(512, Cmax)
        bsizes = (b1, Cmax - b1) if Cmax > b1 else (b1,)
        wslots = (0,) * len(bsizes)
        assignments = []
        for c in range(N_CORES):
            n = int(counts[c])
            cut = min(bsizes[0], n)
            blks = [(c, 0, cut)]
            if len(bsizes) > 1:
                blks.append((c, cut, n))
            assignments.append(blks)

    C = sum(bsizes)
    boffs = np.cumsum((0,) + bsizes)[:-1]

    key = (bsizes, wslots)
    if key not in _compiled:
        _compiled[key] = _build(bsizes, wslots)
    nc = _compiled[key]

    # Pack weights once per expert; cores of a pair share the arrays.
    ug_pack = {}
    dt_pack = {}
    fs_cols = FS * P

    sug_cache = {}
    sdt_cache = {}
    in_maps = []
    for c in range(N_CORES):
        blks = assignments[c]
        experts = [e for e, _, _ in blks]
        for e in experts:
            if e not in ug_pack:
                ug_pack[e] = _pack_ug(w_up[e], w_gate[e])
                dt_pack[e] = _pack_downT(w_down[e])

        q = c % 4
        fh = c // 4
        if fh not in sug_cache:
            cols = slice(fh * fs_cols, (fh + 1) * fs_cols)
            sug_cache[fh] = _pack_ug(sw_up[0][:, cols], sw_gate[0][:, cols])
            sdt_cache[fh] = _pack_downT(sw_down[0][cols, :])

        xe = np.zeros((C, H), np.float32)
        rwb = np.zeros(C, np.float32)
        for (e, lo, hi), boff in zip(blks, boffs):
            n = hi - lo
            xe[boff:boff + n] = x[idx_e[e][lo:hi]]
            rwb[boff:boff + n] = w_e[e][lo:hi]

        wug = np.stack([ug_pack[e] for e in experts], axis=1)
        wdt = np.stack([dt_pack[e] for e in experts], axis=1)

        in_maps.append({
            "xe": _pack_xT(xe),
            "xs": _pack_xT(x[q * ST:(q + 1) * ST]),
            "rwb": np.ascontiguousarray(np.broadcast_to(rwb, (P, C))),
            "wug": np.ascontiguousarray(wug),
            "wdt": np.ascontiguousarray(wdt),
            "sug": sug_cache[fh].reshape(P, 1, FS, 2, KT, P),
            "sdt": sdt_cache[fh],
        })

    trace = bool(int(os.environ.get("KERNEL_TRACE", "0")))
    if trace:
        trace = _try_install_ntff_shim()
    tmpdir = os.environ.get("KERNEL_TRACE_DIR") or None
    res = run_bass_kernel_spmd(
        nc, in_maps, list(range(N_CORES)), trace=trace, tmpdir=tmpdir)
    LAST_EXEC_NS = res.exec_time_ns
    LAST_RESULTS = res

    y = np.zeros((T, H), np.float32)
    for c in range(N_CORES):
        ye = res.results[c]["ye"]          # [HT, P, C]
        yt = ye.transpose(2, 0, 1).reshape(C, H)
        for (e, lo, hi), boff in zip(assignments[c], boffs):
            n = hi - lo
            if n:
                y[idx_e[e][lo:hi]] += yt[boff:boff + n]
        q = c % 4
        ys = res.results[c]["ys"]          # [HT, P, ST]
        y[q * ST:(q + 1) * ST] += ys.transpose(2, 0, 1).reshape(ST, H)

    return y.reshape(2, 1024, H)
